# revision 1
# baseline (speedup 1.0000x reference)
"""GCN (2x GCNConv + global mean pool + FC) on 8 Trainium2 NeuronCores.

Strategy (graph-parallel, dst-sharded):
  - Nodes sharded contiguously across 8 cores (12.5K nodes/core, padded to
    12544 = 98 blocks of 128).
  - Symmetric norm separated: table rows pre-scaled by dinv[src]; drain
    scaled by dinv[dst]; self-loops materialized as real edges.
  - Layer tables (x@W1*dinv, relu(agg1)@W2*dinv) are bf16 node-major rows in
    DRAM, replicated per core via AllGather.
  - Per-edge gather: gpsimd.dma_gather (int16 idxs => table split into
    <=32768-row quarters; edges bucketed by (dst-block, src-quarter)).
  - Scatter: one-hot S ([128 edges x 128 dst-slots], built on DVE via
    iota==slot) matmul'd on the PE, accumulating each dst-block in PSUM.
  - Pool: one-hot graph-membership matmul per node-tile, accumulated in SBUF.
  - Final mean + FC on host (tiny: G=64 x H2=64).
"""
import os
os.environ.setdefault("JAX_PLATFORMS", "cpu")
import sys
if "/opt/trn_rl_repo" not in sys.path:
    sys.path.insert(0, "/opt/trn_rl_repo")
import time
from contextlib import ExitStack

import numpy as np
import ml_dtypes

import concourse.bacc as bacc
import concourse.bass as bass
import concourse.tile as tile
import concourse.mybir as mybir
from concourse.bass_utils import run_bass_kernel_spmd
from concourse.library_config import mlp

NCORES = 8
F = 128          # input / hidden-1 feature width
H2 = 64          # hidden-2 feature width
MAXSEG = 32768   # int16 index range for dma_gather
GROUP_BLOCKS = 8
CBYTES = 256     # dma_gather element bytes (128 bf16)

bf16 = mybir.dt.bfloat16
f32 = mybir.dt.float32
i16 = mybir.dt.int16

LAST_EXEC_S = None
LAST_PREP_S = None
LAST_COMPILE_S = None
LAST_RESULTS = None

_BUILD_CACHE = {}


def _ceil(a, b):
    return -(-a // b)


def _make_groups(nb):
    groups = []
    b = 0
    while b < nb:
        groups.append(list(range(b, min(b + GROUP_BLOCKS, nb))))
        b += GROUP_BLOCKS
    return groups


class Sched:
    """Shared (all-core) stream schedule.

    Edges bucketed by (src-quarter q, dst-block b); bucket capacity =
    max-over-cores count (exact, no rounding). Per (q, group-of-blocks g)
    the buckets are concatenated into a stream padded to x128 only at the
    group end; one dma_gather call covers the group's tiles. A 128-edge
    tile may span adjacent blocks: each (tile, block) pair is a matmul
    "instance" with its own host-built slot column (-1 outside the
    block's range)."""

    def __init__(self, cnt, nb, qn, groups):
        self.nb, self.qn, self.groups = nb, qn, groups
        cap = cnt.max(axis=0).astype(np.int64)      # [qn, nb]
        self.cap = cap
        ng = len(groups)
        self.ct = np.zeros((qn, ng), np.int64)      # tiles per call
        self.gstart = np.zeros((qn, ng), np.int64)  # stream offset (edges)
        self.boff = np.zeros((qn, nb), np.int64)    # block offset in stream
        run = 0
        for q in range(qn):
            for g, blocks in enumerate(groups):
                self.gstart[q, g] = run
                off = 0
                for b in blocks:
                    self.boff[q, b] = run + off
                    off += cap[q, b]
                ctg = _ceil(off, 128)
                self.ct[q, g] = ctg
                run += ctg * 128
        self.ntot = int(run)                        # total stream slots
        self.nt = self.ntot // 128
        # instances: per (g, b): list of (q, t_local, inst_id)
        self.inst = {}
        iid = 0
        for g, blocks in enumerate(groups):
            for b in blocks:
                lst = []
                for q in range(qn):
                    c = int(cap[q, b])
                    if c == 0:
                        continue
                    lo = int(self.boff[q, b] - self.gstart[q, g])
                    hi = lo + c
                    for t in range(lo // 128, _ceil(hi, 128)):
                        lst.append((q, t, iid))
                        iid += 1
                self.inst[(g, b)] = lst
        self.ninst = iid

    def key(self):
        return (self.ntot, self.ninst,
                tuple(self.cap.reshape(-1).tolist()))


def _schedule(dst, q, gi, npc, nb, qn):
    core = dst // npc
    dl = dst % npc
    blk = dl >> 7
    slot = dl & 127
    gi = gi.astype(np.int16)

    key = ((core * qn + q) * nb + blk).astype(np.int64)
    order = np.argsort(key, kind="stable")
    key_s = key[order]
    gi_s = gi[order]
    slot_s = slot[order].astype(np.float32)

    nbuck = NCORES * qn * nb
    cnt = np.bincount(key_s, minlength=nbuck).reshape(NCORES, qn, nb)
    sched = Sched(cnt, nb, qn, _make_groups(nb))

    # place each core's edges at its bucket offset in the shared stream
    bucket_starts = np.zeros(nbuck + 1, np.int64)
    np.cumsum(np.bincount(key_s, minlength=nbuck), out=bucket_starts[1:])
    rank = np.arange(len(key_s)) - bucket_starts[key_s]
    qb = key_s % (qn * nb)
    dest = sched.boff.reshape(-1)[qb] + rank

    gidx_all = np.zeros((NCORES, sched.ntot), np.int16)
    slot_all = np.full((NCORES, sched.ntot), -1.0, np.float32)
    core_s = key_s // (qn * nb)
    gidx_all[core_s, dest] = gi_s
    slot_all[core_s, dest] = slot_s

    # per-instance slot columns [NCORES, 128, ninst]
    cols = np.full((NCORES, 128, sched.ninst), -1.0, np.float32)
    for g, blocks in enumerate(sched.groups):
        for b in blocks:
            for qq, t, iid in sched.inst[(g, b)]:
                base = int(sched.gstart[qq, g]) + t * 128
                lo = int(sched.boff[qq, b])
                hi = lo + int(sched.cap[qq, b])
                pos = np.arange(base, base + 128)
                m = (pos >= lo) & (pos < hi)
                colv = np.where(m, slot_all[:, base : base + 128], -1.0)
                cols[:, :, iid] = colv
    return sched, gidx_all, cols


def _build_bass(npcp, nb, nbA, segs, sched, g_graphs):
    B = nb
    qn = sched.qn
    groups = sched.groups
    nt = sched.nt
    ninst = sched.ninst
    maxct = int(sched.ct.max())
    (qnA, segA), (qnB, segB) = segs
    rows_half = (nbA * 128, (nb - nbA) * 128)
    # quarter -> (half, table row range within that half's AllGather'd table)
    qmap = []
    for qh in range(qnA):
        qmap.append((0, qh * segA, segA))
    for qh in range(qnB):
        qmap.append((1, qh * segB, segB))

    nc = bacc.Bacc("TRN2", num_devices=NCORES, num_swdge_queues=4)
    xin = nc.dram_tensor("xin", [npcp, F], bf16, kind="ExternalInput")
    idxw = nc.dram_tensor("idxw", [128, nt * 8], i16, kind="ExternalInput")
    slots_d = nc.dram_tensor("slots", [128, ninst], f32, kind="ExternalInput")
    dinv_d = nc.dram_tensor("dinvc", [128, B], f32, kind="ExternalInput")
    batch_d = nc.dram_tensor("batchc", [128, B], f32, kind="ExternalInput")
    b1_d = nc.dram_tensor("b1bc", [128, F], f32, kind="ExternalInput")
    iota_d = nc.dram_tensor("iota", [128, 128], bf16, kind="ExternalInput")
    ident_d = nc.dram_tensor("ident", [128, 128], bf16, kind="ExternalInput")
    w1_d = nc.dram_tensor("w1", [F, F], bf16, kind="ExternalInput")
    w2_d = nc.dram_tensor("w2", [F, H2], bf16, kind="ExternalInput")
    out_d = nc.dram_tensor("pooled", [g_graphs, H2], f32, kind="ExternalOutput")

    def mk_tables(pfx):
        locs, ags = [], []
        for h in range(2):
            rh = rows_half[h]
            locs.append(nc.dram_tensor(f"{pfx}_loc{h}", [rh, F], bf16))
            ags.append(
                nc.dram_tensor(
                    f"{pfx}_ag{h}", [rh * NCORES, F], bf16, addr_space="Shared"
                )
            )
        return locs, ags

    tbl1_locs, tbl1_ags = mk_tables("tbl1")
    tbl2_locs, tbl2_ags = mk_tables("tbl2")

    rg = [list(range(NCORES))]

    with tile.TileContext(nc) as tc:
        with ExitStack() as ctx:
            cpool = ctx.enter_context(tc.tile_pool(name="const", bufs=1))
            xtp = ctx.enter_context(tc.tile_pool(name="xt", bufs=3))
            stage = ctx.enter_context(tc.tile_pool(name="stage", bufs=6))
            gbufp = ctx.enter_context(tc.tile_pool(name="gbuf", bufs=12))
            idxp = ctx.enter_context(tc.tile_pool(name="idx", bufs=12))
            sp = ctx.enter_context(tc.tile_pool(name="sp", bufs=4))
            dr = ctx.enter_context(tc.tile_pool(name="dr", bufs=6))
            psum = ctx.enter_context(
                tc.tile_pool(name="psum", bufs=1, space=bass.MemorySpace.PSUM)
            )

            nc.gpsimd.load_library(mlp)

            w1 = cpool.tile([F, F], bf16)
            nc.sync.dma_start(w1[:], w1_d[:])
            w2 = cpool.tile([F, H2], bf16)
            nc.sync.dma_start(w2[:], w2_d[:])
            iota = cpool.tile([128, 128], bf16)
            nc.sync.dma_start(iota[:], iota_d[:])
            ident = cpool.tile([128, 128], bf16)
            nc.sync.dma_start(ident[:], ident_d[:])
            b1bc = cpool.tile([128, F], f32)
            nc.sync.dma_start(b1bc[:], b1_d[:])
            dinvc = cpool.tile([128, B], f32)
            nc.sync.dma_start(dinvc[:], dinv_d[:])
            batchc = cpool.tile([128, B], f32)
            nc.sync.dma_start(batchc[:], batch_d[:])
            slotc = cpool.tile([128, ninst], f32)
            nc.sync.dma_start(slotc[:], slots_d[:])

            pooled_sb = cpool.tile([g_graphs, H2], f32)
            nc.vector.memset(pooled_sb[:], 0.0)

            # ---- phase 0: table1 = (x @ W1) * dinv, AllGather ----
            start = 0
            while start < npcp:
                width = min(512, npcp - start)
                xt = xtp.tile([128, 512], bf16, tag="xt")
                nc.sync.dma_start_transpose(
                    xt[:, 0:width], xin[start : start + width, :]
                )
                for s in range(width // 128):
                    ntile = start // 128 + s
                    pm = psum.tile([128, F], f32, tag="ps", bufs=6)
                    nc.tensor.matmul(
                        pm[:], xt[:, s * 128 : (s + 1) * 128], w1[:],
                        start=True, stop=True,
                    )
                    st = stage.tile([128, F], bf16, tag="mmst")
                    nc.vector.tensor_scalar(
                        st[:], pm[:], dinvc[:, ntile : ntile + 1], None,
                        mybir.AluOpType.mult,
                    )
                    h = 0 if ntile < nbA else 1
                    r0 = ntile * 128 - h * rows_half[0]
                    nc.sync.dma_start(
                        tbl1_locs[h][r0 : r0 + 128, :], st[:]
                    )
                start += width
            for h in range(2):
                nc.gpsimd.collective_compute(
                    "AllGather", mybir.AluOpType.bypass, replica_groups=rg,
                    ins=[tbl1_locs[h][:]], outs=[tbl1_ags[h][:]],
                )

            # ---- layers ----
            def run_layer(tbl_ags_l, tbl_locs_l, outw, drain_fn):
                for gi_, blocks in enumerate(groups):
                    gbs = {}
                    for q in range(qn):
                        ctq = int(sched.ct[q, gi_])
                        if ctq == 0:
                            continue
                        ecol = int(sched.gstart[q, gi_]) // 16
                        it = idxp.tile([128, maxct * 8], i16, tag="it")
                        nc.sync.dma_start(
                            it[:, 0 : ctq * 8],
                            idxw[:, ecol : ecol + ctq * 8],
                        )
                        gb = gbufp.tile([128, maxct, 128], bf16, tag="gb")
                        h, s0, sw = qmap[q]
                        nc.gpsimd.dma_gather(
                            gb[:, 0:ctq, :],
                            tbl_ags_l[h][s0 : s0 + sw, :],
                            it[:, 0 : ctq * 8],
                            ctq * 128, ctq * 128, 128,
                            single_packet=False, queue_num=q % 4,
                        )
                        gbs[q] = gb
                    for b in blocks:
                        insts = sched.inst[(gi_, b)]
                        nmm = len(insts)
                        pm = psum.tile([128, outw], f32, tag="ps", bufs=6)
                        # self-loop term: psum += I^T @ tbl_loc[block]
                        selfrow = dr.tile([128, 128], bf16, tag="selfrow")
                        hb = 0 if b < nbA else 1
                        rb = b * 128 - hb * rows_half[0]
                        nc.sync.dma_start(
                            selfrow[:], tbl_locs_l[hb][rb : rb + 128, :]
                        )
                        nc.tensor.matmul(
                            pm[:], ident[:], selfrow[:, 0:outw],
                            start=True, stop=(nmm == 0),
                        )
                        for i, (q, t, iid) in enumerate(insts):
                            S = sp.tile([128, 128], bf16, tag="S", bufs=32)
                            if iid % 3 == 2:
                                # ACT path: S = relu(1 - |iota - slot|);
                                # exact one-hot, keeps the DVE (which shares
                                # SBUF ports with the Q7 desc-gen) out of
                                # the per-tile critical path.
                                sq = sp.tile([128, 128], bf16, tag="sq", bufs=12)
                                nc.scalar.activation(
                                    sq[:], iota[:],
                                    mybir.ActivationFunctionType.Abs,
                                    bias=slotc[:, iid : iid + 1], scale=-1.0,
                                )
                                nc.scalar.activation(
                                    S[:], sq[:],
                                    mybir.ActivationFunctionType.Relu,
                                    bias=1.0, scale=-1.0,
                                )
                            else:
                                nc.vector.tensor_scalar(
                                    S[:], iota[:], slotc[:, iid : iid + 1],
                                    None, mybir.AluOpType.is_equal,
                                )
                            nc.tensor.matmul(
                                pm[:], S[:], gbs[q][:, t, 0:outw],
                                start=False, stop=(i == nmm - 1),
                            )
                        drain_fn(b, pm)

            def drain1(b, pm):
                t1 = dr.tile([128, F], f32, tag="t1")
                nc.vector.tensor_scalar(
                    t1[:], pm[:], dinvc[:, b : b + 1], None,
                    mybir.AluOpType.mult,
                )
                t2 = dr.tile([128, F], f32, tag="t2")
                nc.vector.tensor_add(t2[:], t1[:], b1bc[:])
                a1 = dr.tile([128, F], bf16, tag="a1")
                nc.scalar.activation(
                    a1[:], t2[:], mybir.ActivationFunctionType.Relu
                )
                # hw2 = relu(agg1) @ W2 * dinv  (transpose agg1 on the PE)
                pt = psum.tile([128, 128], bf16, tag="pst", bufs=2)
                nc.tensor.transpose(pt[:], a1[:], ident[:])
                lt = dr.tile([128, 128], bf16, tag="lt")
                nc.scalar.activation(
                    lt[:], pt[:], mybir.ActivationFunctionType.Copy
                )
                ph = psum.tile([128, H2], f32, tag="ps", bufs=6)
                nc.tensor.matmul(ph[:], lt[:], w2[:], start=True, stop=True)
                hs = stage.tile([128, 128], bf16, tag="hs")
                nc.vector.memset(hs[:, H2:128], 0.0)
                nc.scalar.activation(
                    hs[:, 0:H2], ph[:], mybir.ActivationFunctionType.Copy,
                    scale=dinvc[:, b : b + 1],
                )
                hb = 0 if b < nbA else 1
                rb = b * 128 - hb * rows_half[0]
                nc.sync.dma_start(tbl2_locs[hb][rb : rb + 128, :], hs[:])

            def drain2(b, pm):
                a2 = dr.tile([128, H2], bf16, tag="a2")
                nc.scalar.activation(
                    a2[:], pm[:], mybir.ActivationFunctionType.Copy,
                    scale=dinvc[:, b : b + 1],
                )
                spool = sp.tile([128, g_graphs], bf16, tag="spool")
                nc.vector.tensor_scalar(
                    spool[:], iota[:, 0:g_graphs], batchc[:, b : b + 1],
                    None, mybir.AluOpType.is_equal,
                )
                pp = psum.tile([g_graphs, H2], f32, tag="ps", bufs=6)
                nc.tensor.matmul(pp[:], spool[:], a2[:], start=True, stop=True)
                nc.vector.tensor_add(pooled_sb[:], pooled_sb[:], pp[:])

            stop_phase = int(os.environ.get("K_STOP_PHASE", "3"))
            if stop_phase >= 1:
                run_layer(tbl1_ags, tbl1_locs, F, drain1)
            if stop_phase >= 2:
                for h in range(2):
                    nc.gpsimd.collective_compute(
                        "AllGather", mybir.AluOpType.bypass, replica_groups=rg,
                        ins=[tbl2_locs[h][:]], outs=[tbl2_ags[h][:]],
                    )
            if stop_phase >= 3:
                run_layer(tbl2_ags, tbl2_locs, H2, drain2)

            nc.sync.dma_start(out_d[:], pooled_sb[:])

    nc.compile()
    return nc


def _install_trace_hooks():
    """Register the axon NTFF profile hook (missing antenv.axon_hooks shim)
    and neuter the artifact upload. Dev/profiling only (K_TRACE_DIR)."""
    import types
    import ctypes
    import contextlib

    if "antenv.axon_hooks" in sys.modules:
        return
    lib = ctypes.CDLL("/opt/axon/libaxon_pjrt.so")
    lib.axon_start_nrt_profile.argtypes = [
        ctypes.POINTER(ctypes.c_int64), ctypes.c_size_t,
    ]
    lib.axon_start_nrt_profile.restype = ctypes.c_int64
    lib.axon_stop_nrt_profile.argtypes = [ctypes.c_char_p]
    lib.axon_stop_nrt_profile.restype = ctypes.c_int64

    @contextlib.contextmanager
    def _hook(output_dir, device_ids):
        import jax
        jax.devices()
        if device_ids:
            ids = (ctypes.c_int64 * len(device_ids))(*device_ids)
            rc = lib.axon_start_nrt_profile(ids, len(device_ids))
        else:
            rc = lib.axon_start_nrt_profile(None, 0)
        if rc != 0:
            raise RuntimeError(f"axon_start_nrt_profile rc={rc}")
        try:
            yield
        finally:
            nfiles = lib.axon_stop_nrt_profile(str(output_dir).encode())
            print(f"ntff profile: {nfiles} file(s) -> {output_dir}")

    mod = types.ModuleType("antenv.axon_hooks")
    mod.get_axon_ntff_profile_hook = lambda: _hook
    mod.set_axon_ntff_profile_hook = lambda h: None
    sys.modules["antenv.axon_hooks"] = mod
    import concourse.bass_utils as _bu
    _bu.upload_artifacts = lambda tmpdir: "local://" + str(tmpdir)


def _prep_and_run(x, src, dst, batch, W1, b1, W2, b2, Wfc, bfc, n, g_graphs):
    global LAST_EXEC_S, LAST_PREP_S, LAST_COMPILE_S
    t0 = time.perf_counter()
    npc = n // NCORES
    npcp = _ceil(npc, 128) * 128
    nb = npcp // 128
    # two halves (A = blocks [0, nbA), B = rest), each AllGather'd separately
    # so gathers from half A can start while half B is still being produced
    nbA = _ceil(nb, 2)
    rows_half = (nbA * 128, (nb - nbA) * 128)
    segs = []          # per half: (qn_h, seg_h)
    for rh in rows_half:
        tot_h = rh * NCORES
        qn_h = max(1, _ceil(tot_h, MAXSEG))
        seg_h = _ceil(tot_h, qn_h * 128) * 128
        segs.append((qn_h, seg_h))
    qn = segs[0][0] + segs[1][0]

    deg = (np.bincount(dst, minlength=n) + 1.0).astype(np.float32)
    dinv = (1.0 / np.sqrt(deg)).astype(np.float32)

    # self loops handled in-kernel via an identity matmul per block
    src_all = src
    dst_all = dst
    src_core = src_all // npc
    src_local = src_all % npc
    rowsA = nbA * 128
    half = (src_local >= rowsA).astype(np.int64)
    local_h = src_local - half * rowsA
    rh_arr = np.where(half == 0, rows_half[0], rows_half[1])
    pos_h = src_core * rh_arr + local_h
    # quarter id = (half ? qnA : 0) + pos_h // seg_h ; idx = pos_h % seg_h
    qnA, segA = segs[0]
    qnB, segB = segs[1]
    seg_of = np.where(half == 0, segA, segB)
    src_q = half * qnA + pos_h // seg_of
    src_gi = pos_h % seg_of

    sched, gidx_all, slot_cols = _schedule(
        dst_all, src_q, src_gi, npc, nb, qn
    )
    nt = sched.nt
    if os.environ.get("K_VERBOSE"):
        print(f"schedule: nt={nt} tiles, ninst={sched.ninst}, "
              f"padded_idx={sched.ntot}, "
              f"real_edges/core={len(dst_all) / NCORES:.0f}, "
              f"pad_overhead={sched.ntot * NCORES / len(dst_all) - 1:.1%}")

    # idx arrays wrapped per call slice (each call = one (q, group) stream)
    idx_maps = np.zeros((NCORES, 128, nt * 8), np.int16)
    ct_list = []
    for gi_ in range(len(sched.groups)):
        for q in range(qn):
            if sched.ct[q, gi_]:
                ct_list.append(
                    (int(sched.gstart[q, gi_]) // 128, int(sched.ct[q, gi_]))
                )
    for c in range(NCORES):
        for s0, ln in ct_list:
            seg_idx = gidx_all[c, s0 * 128 : (s0 + ln) * 128]
            w = seg_idx.reshape(-1, 16).T  # [16, ln*8]
            idx_maps[c, :, s0 * 8 : (s0 + ln) * 8] = np.tile(w, (8, 1))

    dinv_pad = np.ones((NCORES, npcp), np.float32)
    batch_pad = np.full((NCORES, npcp), -1.0, np.float32)
    x_pad = np.zeros((NCORES, npcp, F), ml_dtypes.bfloat16)
    for c in range(NCORES):
        lo, hi = c * npc, (c + 1) * npc
        dinv_pad[c, :npc] = dinv[lo:hi]
        batch_pad[c, :npc] = batch[lo:hi].astype(np.float32)
        x_pad[c, :npc] = x[lo:hi].astype(ml_dtypes.bfloat16)
    dinv_cols = dinv_pad.reshape(NCORES, nb, 128).transpose(0, 2, 1).copy()
    batch_cols = batch_pad.reshape(NCORES, nb, 128).transpose(0, 2, 1).copy()

    iota = np.broadcast_to(np.arange(128), (128, 128)).astype(ml_dtypes.bfloat16)
    ident = np.eye(128, dtype=ml_dtypes.bfloat16)
    b1bc = np.broadcast_to(b1, (128, F)).astype(np.float32)
    w1b = W1.astype(ml_dtypes.bfloat16)
    w2b = W2.astype(ml_dtypes.bfloat16)

    LAST_PREP_S = time.perf_counter() - t0

    key = (n, g_graphs, npcp, qn, tuple(segs), sched.key())
    t0 = time.perf_counter()
    if key not in _BUILD_CACHE:
        _BUILD_CACHE.clear()
        _BUILD_CACHE[key] = _build_bass(npcp, nb, nbA, segs, sched, g_graphs)
    nc = _BUILD_CACHE[key]
    LAST_COMPILE_S = time.perf_counter() - t0

    in_maps = []
    for c in range(NCORES):
        in_maps.append(
            {
                "xin": np.ascontiguousarray(x_pad[c]),
                "idxw": np.ascontiguousarray(idx_maps[c]),
                "slots": np.ascontiguousarray(slot_cols[c]),
                "dinvc": np.ascontiguousarray(dinv_cols[c]),
                "batchc": np.ascontiguousarray(batch_cols[c]),
                "b1bc": b1bc,
                "iota": iota,
                "ident": ident,
                "w1": w1b,
                "w2": w2b,
            }
        )
    t0 = time.perf_counter()
    trace_dir = os.environ.get("K_TRACE_DIR")
    if trace_dir:
        _install_trace_hooks()
        res = run_bass_kernel_spmd(
            nc, in_maps, list(range(NCORES)), trace=True, tmpdir=trace_dir
        )
        globals()["LAST_RESULTS"] = res
    else:
        res = run_bass_kernel_spmd(nc, in_maps, list(range(NCORES)))
    LAST_EXEC_S = time.perf_counter() - t0

    pooled = np.zeros((g_graphs, H2), np.float64)
    for c in range(NCORES):
        pooled += res.results[c]["pooled"].astype(np.float64)
    cnt = np.bincount(batch, minlength=g_graphs).astype(np.float64)
    gmean = pooled / np.maximum(cnt, 1.0)[:, None]
    out = (gmean + b2.astype(np.float64)) @ Wfc.astype(np.float64) + bfc.astype(
        np.float64
    )
    return out.astype(np.float32)


def kernel(x, edge_index, batch, W1, b1, W2, b2, Wfc, bfc):
    x = np.asarray(x, dtype=np.float32)
    ei = np.asarray(edge_index)
    src = ei[0].astype(np.int64)
    dst = ei[1].astype(np.int64)
    bat = np.asarray(batch).astype(np.int64)
    n = x.shape[0]
    g_graphs = 64
    return _prep_and_run(
        x, src, dst, bat,
        np.asarray(W1, np.float32), np.asarray(b1, np.float32),
        np.asarray(W2, np.float32), np.asarray(b2, np.float32),
        np.asarray(Wfc, np.float32), np.asarray(bfc, np.float32),
        n, g_graphs,
    )



# revision 3
# speedup vs baseline: 1.0854x; 1.0854x over previous
"""GCN (2x GCNConv + global mean pool + FC) on 8 Trainium2 NeuronCores.

Strategy (graph-parallel, dst-sharded, aggregate-then-transform):
  - Nodes sharded contiguously across 8 cores (12.5K nodes/core, padded to
    12544 = 98 blocks of 128).
  - Layer tables hold PRE-transform rows scaled by dinv[src]: tbl1 = x*dinv
    (host-prescaled, no phase-0 compute), tbl2 = relu(agg1@W1*dinv+b1)*dinv.
    W is applied AFTER aggregation (linearity), so both tables are 128-wide.
  - Tables are AllGather'd in 13 groups of 8 blocks (1024 rows/core), laid
    out core-major per group; 4 int16 quarters = groups [0-3],[4-7],[8-11],
    [12].  Fine AG granularity pipelines table production with gathers and
    kills the inter-layer stall.
  - Per-edge gather: gpsimd.dma_gather (int16 idxs within quarter), streams
    bucketed by (src-quarter q, dst-4-block-group), padded to x128 per call.
  - Aggregation TRANSPOSED on the PE: psum[f, slot] += gb[e,f]^T S[e,slot]
    with gathered tile stationary and the one-hot S moving.  S matrices are
    built 8-at-a-time on the DVE via zero-stride broadcast APs (one
    tensor_tensor is_equal per 8 edge-tiles).
  - Self-loops: psum[f, slot] += selfrow[p,f]^T I[p,slot].
  - Drain: ACT copy -> PE matmul(W) -> (x dinv + b) -> relu -> table / pool.
  - Pool: one-hot graph-membership matmul per node-block, accumulated in
    SBUF; final mean + FC on host (tiny: G=64 x H2=64).
"""
import os
os.environ.setdefault("JAX_PLATFORMS", "cpu")
import sys
if "/opt/trn_rl_repo" not in sys.path:
    sys.path.insert(0, "/opt/trn_rl_repo")
import time
from contextlib import ExitStack

import numpy as np
import ml_dtypes

import concourse.bacc as bacc
import concourse.bass as bass
import concourse.tile as tile
import concourse.mybir as mybir
from concourse.bass_utils import run_bass_kernel_spmd
from concourse.library_config import mlp

NCORES = 8
F = 128          # feature width of both tables (x and h1)
H2 = 64          # hidden-2 feature width
G = 64           # graphs
QROWS = 32768    # int16 quarter size (rows)
TBL_GB = 8       # blocks per AllGather group
DST_GB = 4       # dst blocks per gather-stream group
K = 8            # S matrices built per DVE op

bf16 = mybir.dt.bfloat16
f32 = mybir.dt.float32
i16 = mybir.dt.int16

LAST_EXEC_S = None
LAST_PREP_S = None
LAST_COMPILE_S = None
LAST_RESULTS = None

_BUILD_CACHE = {}


def _ceil(a, b):
    return -(-a // b)


def _tbl_groups(nb):
    """[(local_row0, rows_per_core, ag_row_offset)] per AllGather group."""
    out = []
    off = 0
    b = 0
    while b < nb:
        blocks = min(TBL_GB, nb - b)
        rows = blocks * 128
        out.append((b * 128, rows, off))
        off += rows * NCORES
        b += blocks
    return out


def _make_groups(nb):
    groups = []
    b = 0
    while b < nb:
        groups.append(list(range(b, min(b + DST_GB, nb))))
        b += DST_GB
    return groups


class Sched:
    """Shared (all-core) stream schedule.

    Edges bucketed by (src-quarter q, dst-block b); bucket capacity =
    max-over-cores count (exact, no rounding). Per (q, group-of-blocks g)
    the buckets are concatenated into a stream padded to x128 only at the
    group end; one dma_gather call covers the group's tiles. A 128-edge
    tile may span adjacent blocks: each (tile, block) pair is a matmul
    "instance" with its own host-built slot column (-1 outside the
    block's range)."""

    def __init__(self, cnt, nb, qn, groups):
        self.nb, self.qn, self.groups = nb, qn, groups
        cap = cnt.max(axis=0).astype(np.int64)      # [qn, nb]
        self.cap = cap
        ng = len(groups)
        self.ct = np.zeros((qn, ng), np.int64)      # tiles per call
        self.gstart = np.zeros((qn, ng), np.int64)  # stream offset (edges)
        self.boff = np.zeros((qn, nb), np.int64)    # block offset in stream
        run = 0
        for g, blocks in enumerate(groups):
            for q in range(qn):
                self.gstart[q, g] = run
                off = 0
                for b in blocks:
                    self.boff[q, b] = run + off
                    off += cap[q, b]
                ctg = _ceil(off, 128)
                self.ct[q, g] = ctg
                run += ctg * 128
        self.ntot = int(run)                        # total stream slots
        self.nt = self.ntot // 128
        # instances: per (g, b): list of (q, t_local, iid)
        self.inst = {}
        iid = 0
        for g, blocks in enumerate(groups):
            for b in blocks:
                lst = []
                for q in range(qn):
                    c = int(cap[q, b])
                    if c == 0:
                        continue
                    lo = int(self.boff[q, b] - self.gstart[q, g])
                    hi = lo + c
                    for t in range(lo // 128, _ceil(hi, 128)):
                        lst.append((q, t, iid))
                        iid += 1
                self.inst[(g, b)] = lst
        self.ninst = iid

    def key(self):
        return (self.ntot, self.ninst,
                tuple(self.cap.reshape(-1).tolist()))


def _schedule(dst, q, gi, npc, nb, qn):
    core = dst // npc
    dl = dst % npc
    blk = dl >> 7
    slot = dl & 127
    gi = gi.astype(np.int16)

    key = ((core * qn + q) * nb + blk).astype(np.int64)
    order = np.argsort(key, kind="stable")
    key_s = key[order]
    gi_s = gi[order]
    slot_s = slot[order].astype(np.float32)

    nbuck = NCORES * qn * nb
    cnt = np.bincount(key_s, minlength=nbuck).reshape(NCORES, qn, nb)
    sched = Sched(cnt, nb, qn, _make_groups(nb))

    # place each core's edges at its bucket offset in the shared stream
    bucket_starts = np.zeros(nbuck + 1, np.int64)
    np.cumsum(np.bincount(key_s, minlength=nbuck), out=bucket_starts[1:])
    rank = np.arange(len(key_s)) - bucket_starts[key_s]
    qb = key_s % (qn * nb)
    dest = sched.boff.reshape(-1)[qb] + rank

    gidx_all = np.zeros((NCORES, sched.ntot), np.int16)
    slot_all = np.full((NCORES, sched.ntot), -1.0, np.float32)
    core_s = key_s // (qn * nb)
    gidx_all[core_s, dest] = gi_s
    slot_all[core_s, dest] = slot_s

    # per-instance slot columns [NCORES, 128, ninst]
    cols = np.full((NCORES, 128, sched.ninst), -1.0, np.float32)
    for g, blocks in enumerate(sched.groups):
        for b in blocks:
            for qq, t, iid in sched.inst[(g, b)]:
                base = int(sched.gstart[qq, g]) + t * 128
                lo = int(sched.boff[qq, b])
                hi = lo + int(sched.cap[qq, b])
                pos = np.arange(base, base + 128)
                m = (pos >= lo) & (pos < hi)
                colv = np.where(m, slot_all[:, base : base + 128], -1.0)
                cols[:, :, iid] = colv
    return sched, gidx_all, cols


def _build_bass(npcp, nb, qrows, sched, ninst_pad, maxct, g_graphs):
    qn = sched.qn
    groups = sched.groups
    nt = sched.nt
    trows = sum(qrows)
    tgroups = _tbl_groups(nb)

    nc = bacc.Bacc("TRN2", num_devices=NCORES, num_swdge_queues=4)
    xin = nc.dram_tensor("xin", [npcp, F], bf16, kind="ExternalInput")
    idxw = nc.dram_tensor("idxw", [128, nt * 8], i16, kind="ExternalInput")
    slots_d = nc.dram_tensor("slots", [128, ninst_pad], bf16, kind="ExternalInput")
    dinv_d = nc.dram_tensor("dinvc", [128, nb], f32, kind="ExternalInput")
    batch_d = nc.dram_tensor("batchc", [128, nb], f32, kind="ExternalInput")
    b1_d = nc.dram_tensor("b1bc", [128, F], f32, kind="ExternalInput")
    iota_d = nc.dram_tensor("iota", [128, 128], bf16, kind="ExternalInput")
    ident_d = nc.dram_tensor("ident", [128, 128], bf16, kind="ExternalInput")
    w1_d = nc.dram_tensor("w1", [F, F], bf16, kind="ExternalInput")
    w2_d = nc.dram_tensor("w2", [F, H2], bf16, kind="ExternalInput")
    out_d = nc.dram_tensor("pooled", [g_graphs, H2], f32, kind="ExternalOutput")

    tbl1_loc = nc.dram_tensor("tbl1_loc", [npcp, F], bf16)
    tbl1_ag = nc.dram_tensor("tbl1_ag", [trows, F], bf16, addr_space="Shared")
    tbl2_loc = nc.dram_tensor("tbl2_loc", [npcp, F], bf16)
    tbl2_ag = nc.dram_tensor("tbl2_ag", [trows, F], bf16, addr_space="Shared")

    rg = [list(range(NCORES))]

    def emit_ag(loc, ag):
        for l0, rows, off in tgroups:
            nc.gpsimd.collective_compute(
                "AllGather", mybir.AluOpType.bypass, replica_groups=rg,
                ins=[loc[l0 : l0 + rows, :]],
                outs=[ag[off : off + rows * NCORES, :]],
            )

    with tile.TileContext(nc) as tc:
        with ExitStack() as ctx:
            cpool = ctx.enter_context(tc.tile_pool(name="const", bufs=1))
            stage = ctx.enter_context(tc.tile_pool(name="stage", bufs=6))
            gbufp = ctx.enter_context(tc.tile_pool(name="gbuf", bufs=9))
            idxp = ctx.enter_context(tc.tile_pool(name="idx", bufs=9))
            sp = ctx.enter_context(tc.tile_pool(name="sp", bufs=4))
            dr = ctx.enter_context(tc.tile_pool(name="dr", bufs=6))
            psum = ctx.enter_context(
                tc.tile_pool(name="psum", bufs=1, space=bass.MemorySpace.PSUM)
            )

            nc.gpsimd.load_library(mlp)

            # table 1 = prescaled x: DRAM copy + chunked AllGather, ASAP
            for l0, rows, off in tgroups:
                nc.sync.dma_start(
                    tbl1_loc[l0 : l0 + rows, :], xin[l0 : l0 + rows, :]
                )
            emit_ag(tbl1_loc, tbl1_ag)

            w1 = cpool.tile([F, F], bf16)
            nc.sync.dma_start(w1[:], w1_d[:])
            w2 = cpool.tile([F, H2], bf16)
            nc.sync.dma_start(w2[:], w2_d[:])
            iota = cpool.tile([128, 128], bf16)
            nc.sync.dma_start(iota[:], iota_d[:])
            ident = cpool.tile([128, 128], bf16)
            nc.sync.dma_start(ident[:], ident_d[:])
            b1bc = cpool.tile([128, F], f32)
            nc.sync.dma_start(b1bc[:], b1_d[:])
            dinvc = cpool.tile([128, nb], f32)
            nc.sync.dma_start(dinvc[:], dinv_d[:])
            batchc = cpool.tile([128, nb], f32)
            nc.sync.dma_start(batchc[:], batch_d[:])
            slotc = cpool.tile([128, ninst_pad], bf16)
            nc.sync.dma_start(slotc[:], slots_d[:])

            pooled_sb = cpool.tile([g_graphs, H2], f32)
            nc.vector.memset(pooled_sb[:], 0.0)

            qcount = [0]

            def run_layer(tbl_ag_l, self_dram, drain_fn):
                sbuilt = {}
                for sg, blocks in enumerate(groups):
                    gbs = {}
                    for q in range(qn):
                        ctq = int(sched.ct[q, sg])
                        if ctq == 0:
                            continue
                        ecol = int(sched.gstart[q, sg]) // 16
                        it = idxp.tile([128, maxct * 8], i16, tag="it")
                        nc.sync.dma_start(
                            it[:, 0 : ctq * 8],
                            idxw[:, ecol : ecol + ctq * 8],
                        )
                        gb = gbufp.tile([128, maxct, 128], bf16, tag="gb")
                        nc.gpsimd.dma_gather(
                            gb[:, 0:ctq, :],
                            tbl_ag_l[q * QROWS : q * QROWS + qrows[q], :],
                            it[:, 0 : ctq * 8],
                            ctq * 128, ctq * 128, 128,
                            single_packet=False,
                            queue_num=qcount[0] % 4,
                        )
                        qcount[0] += 1
                        gbs[q] = gb
                    for b in blocks:
                        insts = sched.inst[(sg, b)]
                        nmm = len(insts)
                        pm = psum.tile([128, 128], f32, tag="agg", bufs=5)
                        # self-loop: psum[f, slot] += selfrow^T
                        selfrow = dr.tile([128, 128], bf16, tag="selfrow")
                        nc.sync.dma_start(
                            selfrow[:], self_dram[b * 128 : (b + 1) * 128, :]
                        )
                        nc.tensor.matmul(
                            pm[:], selfrow[:], ident[:],
                            start=True, stop=(nmm == 0),
                        )
                        for i, (q, t, iid) in enumerate(insts):
                            kb = iid // K
                            if kb not in sbuilt:
                                S = sp.tile([128, K * 128], bf16, tag="S",
                                            bufs=12)
                                s3 = S[:].rearrange("p (k c) -> p k c", k=K)
                                nc.vector.tensor_tensor(
                                    s3,
                                    iota[:].unsqueeze(1).to_broadcast(
                                        (128, K, 128)),
                                    slotc[:, kb * K : (kb + 1) * K]
                                    .unsqueeze(2).to_broadcast((128, K, 128)),
                                    mybir.AluOpType.is_equal,
                                )
                                sbuilt[kb] = S
                            o = (iid % K) * 128
                            nc.tensor.matmul(
                                pm[:], gbs[q][:, t, :],
                                sbuilt[kb][:, o : o + 128],
                                start=False, stop=(i == nmm - 1),
                            )
                        drain_fn(b, pm)

            def drain1(b, pm):
                aggT = dr.tile([128, 128], bf16, tag="aggT")
                nc.scalar.activation(
                    aggT[:], pm[:], mybir.ActivationFunctionType.Copy
                )
                pz = psum.tile([128, F], f32, tag="pz", bufs=2)
                nc.tensor.matmul(pz[:], aggT[:], w1[:], start=True, stop=True)
                u = dr.tile([128, F], f32, tag="u")
                nc.vector.scalar_tensor_tensor(
                    u[:], pz[:], dinvc[:, b : b + 1], b1bc[:],
                    mybir.AluOpType.mult, mybir.AluOpType.add,
                )
                hs = stage.tile([128, F], bf16, tag="hs")
                nc.scalar.activation(
                    hs[:], u[:], mybir.ActivationFunctionType.Relu,
                    scale=dinvc[:, b : b + 1],
                )
                nc.sync.dma_start(tbl2_loc[b * 128 : (b + 1) * 128, :], hs[:])

            def drain2(b, pm):
                a2T = dr.tile([128, 128], bf16, tag="aggT")
                nc.scalar.activation(
                    a2T[:], pm[:], mybir.ActivationFunctionType.Copy
                )
                pz = psum.tile([128, H2], f32, tag="pz", bufs=2)
                nc.tensor.matmul(pz[:], a2T[:], w2[:], start=True, stop=True)
                a2 = dr.tile([128, H2], bf16, tag="a2")
                nc.scalar.activation(
                    a2[:], pz[:], mybir.ActivationFunctionType.Copy,
                    scale=dinvc[:, b : b + 1],
                )
                spool = sp.tile([128, g_graphs], bf16, tag="spool")
                nc.vector.tensor_scalar(
                    spool[:], iota[:, 0:g_graphs], batchc[:, b : b + 1],
                    None, mybir.AluOpType.is_equal,
                )
                pp = psum.tile([g_graphs, H2], f32, tag="pp", bufs=1)
                nc.tensor.matmul(pp[:], spool[:], a2[:], start=True, stop=True)
                nc.vector.tensor_add(pooled_sb[:], pooled_sb[:], pp[:])

            run_layer(tbl1_ag, xin, drain1)
            emit_ag(tbl2_loc, tbl2_ag)
            run_layer(tbl2_ag, tbl2_loc, drain2)

            nc.sync.dma_start(out_d[:], pooled_sb[:])

    nc.compile()
    return nc


def _install_trace_hooks():
    """Register the axon NTFF profile hook (missing antenv.axon_hooks shim)
    and neuter the artifact upload. Dev/profiling only (K_TRACE_DIR)."""
    import types
    import ctypes
    import contextlib

    if "antenv.axon_hooks" in sys.modules:
        return
    lib = ctypes.CDLL("/opt/axon/libaxon_pjrt.so")
    lib.axon_start_nrt_profile.argtypes = [
        ctypes.POINTER(ctypes.c_int64), ctypes.c_size_t,
    ]
    lib.axon_start_nrt_profile.restype = ctypes.c_int64
    lib.axon_stop_nrt_profile.argtypes = [ctypes.c_char_p]
    lib.axon_stop_nrt_profile.restype = ctypes.c_int64

    @contextlib.contextmanager
    def _hook(output_dir, device_ids):
        import jax
        jax.devices()
        if device_ids:
            ids = (ctypes.c_int64 * len(device_ids))(*device_ids)
            rc = lib.axon_start_nrt_profile(ids, len(device_ids))
        else:
            rc = lib.axon_start_nrt_profile(None, 0)
        if rc != 0:
            raise RuntimeError(f"axon_start_nrt_profile rc={rc}")
        try:
            yield
        finally:
            nfiles = lib.axon_stop_nrt_profile(str(output_dir).encode())
            print(f"ntff profile: {nfiles} file(s) -> {output_dir}")

    mod = types.ModuleType("antenv.axon_hooks")
    mod.get_axon_ntff_profile_hook = lambda: _hook
    mod.set_axon_ntff_profile_hook = lambda h: None
    sys.modules["antenv.axon_hooks"] = mod
    import concourse.bass_utils as _bu
    _bu.upload_artifacts = lambda tmpdir: "local://" + str(tmpdir)


def _prep_and_run(x, src, dst, batch, W1, b1, W2, b2, Wfc, bfc, n, g_graphs):
    global LAST_EXEC_S, LAST_PREP_S, LAST_COMPILE_S
    t0 = time.perf_counter()
    npc = n // NCORES
    npcp = _ceil(npc, 128) * 128
    nb = npcp // 128
    qn = 4

    deg = (np.bincount(dst, minlength=n) + 1.0).astype(np.float32)
    dinv = (1.0 / np.sqrt(deg)).astype(np.float32)

    # table position of src node: AG-group layout, core-major per group
    src_core = src // npc
    src_local = src % npc
    gi_g = (src_local >> 7) // TBL_GB
    tgroups = _tbl_groups(nb)
    rows_pc = np.array([r for _, r, _ in tgroups], np.int64)
    offs = np.array([o for _, _, o in tgroups], np.int64)
    l0s = np.array([l for l, _, _ in tgroups], np.int64)
    pos = offs[gi_g] + src_core * rows_pc[gi_g] + (src_local - l0s[gi_g])
    src_q = pos // QROWS
    src_gi = pos % QROWS
    qrows = []
    trows = sum(int(r) * NCORES for _, r, _ in tgroups)
    for q in range(qn):
        qrows.append(min(QROWS, max(0, trows - q * QROWS)))

    sched, gidx_all, slot_cols = _schedule(dst, src_q, src_gi, npc, nb, qn)
    nt = sched.nt
    maxct = int(sched.ct.max())
    ninst_pad = _ceil(sched.ninst, K) * K
    if os.environ.get("K_VERBOSE"):
        print(f"schedule: nt={nt} tiles, ninst={sched.ninst}, maxct={maxct}, "
              f"padded_idx={sched.ntot}, "
              f"real_edges/core={len(dst) / NCORES:.0f}, "
              f"pad_overhead={sched.ntot * NCORES / len(dst) - 1:.1%}")

    # idx arrays wrapped per call slice (each call = one (q, group) stream)
    idx_maps = np.zeros((NCORES, 128, nt * 8), np.int16)
    ct_list = []
    for gi_ in range(len(sched.groups)):
        for q in range(qn):
            if sched.ct[q, gi_]:
                ct_list.append(
                    (int(sched.gstart[q, gi_]) // 128, int(sched.ct[q, gi_]))
                )
    for c in range(NCORES):
        for s0, ln in ct_list:
            seg_idx = gidx_all[c, s0 * 128 : (s0 + ln) * 128]
            w = seg_idx.reshape(-1, 16).T  # [16, ln*8]
            idx_maps[c, :, s0 * 8 : (s0 + ln) * 8] = np.tile(w, (8, 1))

    slots_pad = np.full((NCORES, 128, ninst_pad), -1.0, np.float32)
    slots_pad[:, :, : sched.ninst] = slot_cols

    dinv_pad = np.ones((NCORES, npcp), np.float32)
    batch_pad = np.full((NCORES, npcp), -1.0, np.float32)
    x_pad = np.zeros((NCORES, npcp, F), ml_dtypes.bfloat16)
    xs = x * dinv[:, None]
    for c in range(NCORES):
        lo, hi = c * npc, (c + 1) * npc
        dinv_pad[c, :npc] = dinv[lo:hi]
        batch_pad[c, :npc] = batch[lo:hi].astype(np.float32)
        x_pad[c, :npc] = xs[lo:hi].astype(ml_dtypes.bfloat16)
    dinv_cols = dinv_pad.reshape(NCORES, nb, 128).transpose(0, 2, 1).copy()
    batch_cols = batch_pad.reshape(NCORES, nb, 128).transpose(0, 2, 1).copy()

    iota = np.broadcast_to(np.arange(128), (128, 128)).astype(ml_dtypes.bfloat16)
    ident = np.eye(128, dtype=ml_dtypes.bfloat16)
    b1bc = np.broadcast_to(b1, (128, F)).astype(np.float32)
    w1b = W1.astype(ml_dtypes.bfloat16)
    w2b = W2.astype(ml_dtypes.bfloat16)

    LAST_PREP_S = time.perf_counter() - t0

    key = (n, g_graphs, npcp, qn, tuple(qrows), sched.key())
    t0 = time.perf_counter()
    if key not in _BUILD_CACHE:
        _BUILD_CACHE.clear()
        _BUILD_CACHE[key] = _build_bass(
            npcp, nb, qrows, sched, ninst_pad, maxct, g_graphs
        )
    nc = _BUILD_CACHE[key]
    LAST_COMPILE_S = time.perf_counter() - t0

    in_maps = []
    for c in range(NCORES):
        in_maps.append(
            {
                "xin": np.ascontiguousarray(x_pad[c]),
                "idxw": np.ascontiguousarray(idx_maps[c]),
                "slots": np.ascontiguousarray(
                    slots_pad[c].astype(ml_dtypes.bfloat16)
                ),
                "dinvc": np.ascontiguousarray(dinv_cols[c]),
                "batchc": np.ascontiguousarray(batch_cols[c]),
                "b1bc": b1bc,
                "iota": iota,
                "ident": ident,
                "w1": w1b,
                "w2": w2b,
            }
        )
    t0 = time.perf_counter()
    trace_dir = os.environ.get("K_TRACE_DIR")
    if trace_dir:
        _install_trace_hooks()
        res = run_bass_kernel_spmd(
            nc, in_maps, list(range(NCORES)), trace=True, tmpdir=trace_dir
        )
        globals()["LAST_RESULTS"] = res
    else:
        res = run_bass_kernel_spmd(nc, in_maps, list(range(NCORES)))
    LAST_EXEC_S = time.perf_counter() - t0

    pooled = np.zeros((g_graphs, H2), np.float64)
    for c in range(NCORES):
        pooled += res.results[c]["pooled"].astype(np.float64)
    cnt = np.bincount(batch, minlength=g_graphs).astype(np.float64)
    gmean = pooled / np.maximum(cnt, 1.0)[:, None]
    out = (gmean + b2.astype(np.float64)) @ Wfc.astype(np.float64) + bfc.astype(
        np.float64
    )
    return out.astype(np.float32)


def kernel(x, edge_index, batch, W1, b1, W2, b2, Wfc, bfc):
    x = np.asarray(x, dtype=np.float32)
    ei = np.asarray(edge_index)
    src = ei[0].astype(np.int64)
    dst = ei[1].astype(np.int64)
    bat = np.asarray(batch).astype(np.int64)
    n = x.shape[0]
    g_graphs = 64
    return _prep_and_run(
        x, src, dst, bat,
        np.asarray(W1, np.float32), np.asarray(b1, np.float32),
        np.asarray(W2, np.float32), np.asarray(b2, np.float32),
        np.asarray(Wfc, np.float32), np.asarray(bfc, np.float32),
        n, g_graphs,
    )


# revision 4
# speedup vs baseline: 1.1138x; 1.0261x over previous
"""GCN (2x GCNConv + global mean pool + FC) on 8 Trainium2 NeuronCores.

Strategy (graph-parallel, dst-sharded, aggregate-then-transform):
  - Nodes sharded contiguously across 8 cores (12.5K nodes/core, padded to
    12544 = 98 blocks of 128).
  - Layer tables hold PRE-transform rows scaled by dinv[src]: tbl1 = x*dinv
    (host-prescaled, uploaded FULL per core in gather layout -> no phase-0
    compute and no tbl1 collectives), tbl2 = relu(agg1@W1*dinv+b1)*dinv.
    W is applied AFTER aggregation (linearity), so both tables are 128-wide.
  - tbl2 is AllGather'd in 10 groups (9x10 + 1x8 blocks, core-major rows per
    group) pipelined with production; int16 quarters = 3 groups each (30720
    rows) + group 9, with a memset zero row in each quarter's tail gap.
  - Per-edge gather: gpsimd.dma_gather (int16 idxs within quarter), streams
    bucketed by (src-quarter q, dst-4-block-group).  HYBRID layout: per
    (q<3, dst-block) the first 4 copies of each dst slot sit at their slot
    position in 4 "aligned" tiles (pad gathers the zero row), leftovers and
    all of q3 go to dense tiles.
  - Aggregation TRANSPOSED on the PE: psum[f, slot] += gb[e,f]^T M[e,slot];
    M = identity (constant) for aligned tiles, a one-hot S for dense tiles.
    S matrices are built 8-at-a-time on the DVE via zero-stride broadcast
    APs (one tensor_tensor is_equal per 8 dense tiles).
  - Self-loops: psum[f, slot] += selfrow[p,f]^T I[p,slot].
  - Drain: ACT copy -> PE matmul(W) -> (x dinv + b) -> relu -> table / pool.
  - Pool: one-hot graph-membership matmul per node-block, accumulated in
    SBUF; final mean + FC on host (tiny: G=64 x H2=64).
"""
import os
os.environ.setdefault("JAX_PLATFORMS", "cpu")
import sys
if "/opt/trn_rl_repo" not in sys.path:
    sys.path.insert(0, "/opt/trn_rl_repo")
import time
from contextlib import ExitStack

import numpy as np
import ml_dtypes

import concourse.bacc as bacc
import concourse.bass as bass
import concourse.tile as tile
import concourse.mybir as mybir
from concourse.bass_utils import run_bass_kernel_spmd
from concourse.library_config import mlp

NCORES = 8
F = 128          # feature width of both tables (x and h1)
H2 = 64          # hidden-2 feature width
G = 64           # graphs
QROWS = 32768    # int16 quarter size (rows)
QN = 4
DST_GB = 4       # dst blocks per gather-stream group
K = 8            # S matrices built per DVE op
ALIGN_T = 4      # aligned tiles per (q<3, dst block)
ALLOC_ROWS = QN * QROWS

bf16 = mybir.dt.bfloat16
f32 = mybir.dt.float32
i16 = mybir.dt.int16

LAST_EXEC_S = None
LAST_PREP_S = None
LAST_COMPILE_S = None
LAST_RESULTS = None

_BUILD_CACHE = {}


def _ceil(a, b):
    return -(-a // b)


# ---- table layouts (npcp = 12544 = 98 blocks) ----
# tbl1 (input x, no collective): quarter-core-major.  Quarter q<3 holds
# local rows [q*3840, (q+1)*3840) of every core; q3 holds [11520, 12544).
T1_QL0 = [0, 3840, 7680, 11520]
T1_QRPC = [3840, 3840, 3840, 1024]
# tbl2 (AllGather per group): 9 groups of 1280 rows + 1 of 1024, core-major
# per group.  Quarter q<3 = groups [3q, 3q+3) at offsets {0,10240,20480}.
T2_GL0 = [1280 * i for i in range(9)] + [11520]
T2_GRPC = [1280] * 9 + [1024]
T2_GOFF = [0, 10240, 20480, 32768, 43008, 53248, 65536, 75776, 86016, 98304]
ZROW = [30720, 30720, 30720, 8192]   # zero-row idx within each quarter


def _make_groups(nb):
    groups = []
    b = 0
    while b < nb:
        groups.append(list(range(b, min(b + DST_GB, nb))))
        b += DST_GB
    return groups


class Sched:
    """Hybrid stream schedule (shared across cores).

    Per (q, dst-group sg): ALIGN_T aligned tiles per block (q<3) followed
    by dense tiles holding per-(q,b) leftover buckets (capacity = max over
    cores), padded to x128 at the (q,sg) end."""

    def __init__(self, dcnt, nb, groups):
        self.nb, self.groups = nb, groups
        cap = dcnt.max(axis=0).astype(np.int64)      # [QN, nb] dense caps
        self.cap = cap
        ng = len(groups)
        self.aq = [ALIGN_T, ALIGN_T, ALIGN_T, 0]
        self.ct = np.zeros((QN, ng), np.int64)       # tiles per call
        self.gstart = np.zeros((QN, ng), np.int64)   # stream offset (edges)
        self.boff = np.zeros((QN, nb), np.int64)     # dense block offset
        run = 0
        for g, blocks in enumerate(groups):
            for q in range(QN):
                self.gstart[q, g] = run
                na = self.aq[q] * len(blocks)
                off = na * 128
                for b in blocks:
                    self.boff[q, b] = run + off
                    off += cap[q, b]
                ctg = na + _ceil(off - na * 128, 128)
                self.ct[q, g] = ctg
                run += ctg * 128
        self.ntot = int(run)
        self.nt = self.ntot // 128
        # dense instances: per (g, b): list of (q, t_local, iid)
        self.inst = {}
        iid = 0
        for g, blocks in enumerate(groups):
            for b in blocks:
                lst = []
                for q in range(QN):
                    c = int(cap[q, b])
                    if c == 0:
                        continue
                    lo = int(self.boff[q, b] - self.gstart[q, g])
                    hi = lo + c
                    for t in range(lo // 128, _ceil(hi, 128)):
                        lst.append((q, t, iid))
                        iid += 1
                self.inst[(g, b)] = lst
        self.ninst = iid

    def key(self):
        return (self.ntot, self.ninst,
                tuple(self.cap.reshape(-1).tolist()))


def _schedule(src, dst, npc, nb):
    """Returns sched, gidx1, gidx2 [NCORES, ntot] i16, cols [NCORES,128,ninst]."""
    cd = dst // npc
    dl = dst % npc
    blk = dl >> 7
    slot = dl & 127
    cs = src // npc
    l = src % npc
    q = np.minimum(l // 3840, 3)
    idx1 = (cs * np.take(T1_QRPC, q) + (l - np.take(T1_QL0, q))).astype(np.int16)
    gi = np.minimum(l // 1280, 9)
    pos2 = (np.take(T2_GOFF, gi) + cs * np.take(T2_GRPC, gi)
            + (l - np.take(T2_GL0, gi)))
    idx2 = (pos2 - q * QROWS).astype(np.int16)

    key = ((cd * QN + q) * nb + blk).astype(np.int64)
    order = np.lexsort((slot, key))
    ks = key[order]
    ss = slot[order]
    i1s = idx1[order]
    i2s = idx2[order]
    n = len(ks)

    # rank within (bucket, slot)
    grp = ks * 128 + ss
    newg = np.empty(n, bool); newg[0] = True
    newg[1:] = grp[1:] != grp[:-1]
    gfirst = np.maximum.accumulate(np.where(newg, np.arange(n), 0))
    rank = np.arange(n) - gfirst

    qs = (ks // nb) % QN
    aq_s = np.where(qs < 3, ALIGN_T, 0)
    aligned = rank < aq_s

    # dense rank within bucket
    didx = np.nonzero(~aligned)[0]
    dk = ks[didx]
    newk = np.empty(len(dk), bool)
    if len(dk):
        newk[0] = True
        newk[1:] = dk[1:] != dk[:-1]
    dfirst = np.maximum.accumulate(np.where(newk, np.arange(len(dk)), 0))
    drank = np.arange(len(dk)) - dfirst

    nbuck = NCORES * QN * nb
    dcnt = np.bincount(dk, minlength=nbuck).reshape(NCORES, QN, nb)
    sched = Sched(dcnt, nb, _make_groups(nb))

    # stream positions
    sg_s = blk[order] // DST_GB
    bi_s = blk[order] - sg_s * DST_GB
    gst = sched.gstart[qs, sg_s]
    pos = np.empty(n, np.int64)
    am = aligned
    pos[am] = gst[am] + (bi_s[am] * ALIGN_T + rank[am]) * 128 + ss[am]
    pos[didx] = sched.boff.reshape(-1)[dk % (QN * nb)] + drank

    # default idx = quarter zero row; slot = -1
    zr1 = np.zeros(sched.ntot, np.int16)
    for g in range(len(sched.groups)):
        for qq in range(QN):
            s0 = int(sched.gstart[qq, g])
            s1 = s0 + int(sched.ct[qq, g]) * 128
            zr1[s0:s1] = ZROW[qq]
    gidx1 = np.broadcast_to(zr1, (NCORES, sched.ntot)).copy()
    gidx2 = gidx1.copy()
    slot_all = np.full((NCORES, sched.ntot), -1.0, np.float32)
    core_s = ks // (QN * nb)
    gidx1[core_s, pos] = i1s
    gidx2[core_s, pos] = i2s
    slot_all[core_s[didx], pos[didx]] = ss[didx].astype(np.float32)

    # per-dense-instance slot columns
    cols = np.full((NCORES, 128, sched.ninst), -1.0, np.float32)
    for g, blocks in enumerate(sched.groups):
        for b in blocks:
            for qq, t, iid in sched.inst[(g, b)]:
                base = int(sched.gstart[qq, g]) + t * 128
                lo = int(sched.boff[qq, b])
                hi = lo + int(sched.cap[qq, b])
                p = np.arange(base, base + 128)
                m = (p >= lo) & (p < hi)
                cols[:, :, iid] = np.where(
                    m, slot_all[:, base : base + 128], -1.0)
    return sched, gidx1, gidx2, cols


def _wrap_idx(sched, gidx):
    """[NCORES, 128, nt*8] wrapped (x16) + replicated (x8) idx layout."""
    nt = sched.nt
    out = np.zeros((NCORES, 128, nt * 8), np.int16)
    calls = []
    for g in range(len(sched.groups)):
        for q in range(QN):
            if sched.ct[q, g]:
                calls.append((int(sched.gstart[q, g]) // 128,
                              int(sched.ct[q, g])))
    for c in range(NCORES):
        for s0, ln in calls:
            seg = gidx[c, s0 * 128 : (s0 + ln) * 128]
            w = seg.reshape(-1, 16).T
            out[c, :, s0 * 8 : (s0 + ln) * 8] = np.tile(w, (8, 1))
    return out


def _build_bass(npcp, nb, sched, ninst_pad, maxct, g_graphs):
    groups = sched.groups
    nt = sched.nt

    nc = bacc.Bacc("TRN2", num_devices=NCORES, num_swdge_queues=4)
    xtbl = nc.dram_tensor("xtbl", [ALLOC_ROWS, F], bf16, kind="ExternalInput")
    xself = nc.dram_tensor("xself", [npcp, F], bf16, kind="ExternalInput")
    idx1_d = nc.dram_tensor("idx1", [128, nt * 8], i16, kind="ExternalInput")
    idx2_d = nc.dram_tensor("idx2", [128, nt * 8], i16, kind="ExternalInput")
    slots_d = nc.dram_tensor("slots", [128, ninst_pad], bf16, kind="ExternalInput")
    dinv_d = nc.dram_tensor("dinvc", [128, nb], f32, kind="ExternalInput")
    batch_d = nc.dram_tensor("batchc", [128, nb], f32, kind="ExternalInput")
    b1_d = nc.dram_tensor("b1bc", [128, F], f32, kind="ExternalInput")
    iota_d = nc.dram_tensor("iota", [128, 128], bf16, kind="ExternalInput")
    ident_d = nc.dram_tensor("ident", [128, 128], bf16, kind="ExternalInput")
    w1_d = nc.dram_tensor("w1", [F, F], bf16, kind="ExternalInput")
    w2_d = nc.dram_tensor("w2", [F, H2], bf16, kind="ExternalInput")
    out_d = nc.dram_tensor("pooled", [g_graphs, H2], f32, kind="ExternalOutput")

    tbl2_loc = nc.dram_tensor("tbl2_loc", [npcp, F], bf16)
    tbl2_ag = nc.dram_tensor("tbl2_ag", [ALLOC_ROWS, F], bf16,
                             addr_space="Shared")
    rg = [list(range(NCORES))]

    with tile.TileContext(nc) as tc:
        with ExitStack() as ctx:
            cpool = ctx.enter_context(tc.tile_pool(name="const", bufs=1))
            stage = ctx.enter_context(tc.tile_pool(name="stage", bufs=6))
            gbufp = ctx.enter_context(tc.tile_pool(name="gbuf", bufs=8))
            sp = ctx.enter_context(tc.tile_pool(name="sp", bufs=4))
            dr = ctx.enter_context(tc.tile_pool(name="dr", bufs=6))
            psum = ctx.enter_context(
                tc.tile_pool(name="psum", bufs=1, space=bass.MemorySpace.PSUM)
            )

            nc.gpsimd.load_library(mlp)

            # zero rows in tbl2_ag quarter gaps
            zrow = cpool.tile([1, F], bf16)
            nc.vector.memset(zrow[:], 0.0)
            for q in range(QN):
                r = q * QROWS + ZROW[q]
                nc.sync.dma_start(tbl2_ag[r : r + 1, :], zrow[:])

            w1 = cpool.tile([F, F], bf16)
            nc.sync.dma_start(w1[:], w1_d[:])
            w2 = cpool.tile([F, H2], bf16)
            nc.sync.dma_start(w2[:], w2_d[:])
            iota = cpool.tile([128, 128], bf16)
            nc.sync.dma_start(iota[:], iota_d[:])
            ident = cpool.tile([128, 128], bf16)
            nc.sync.dma_start(ident[:], ident_d[:])
            b1bc = cpool.tile([128, F], f32)
            nc.sync.dma_start(b1bc[:], b1_d[:])
            dinvc = cpool.tile([128, nb], f32)
            nc.sync.dma_start(dinvc[:], dinv_d[:])
            batchc = cpool.tile([128, nb], f32)
            nc.sync.dma_start(batchc[:], batch_d[:])
            slotc = cpool.tile([128, ninst_pad], bf16)
            nc.sync.dma_start(slotc[:], slots_d[:])
            idx1_sb = cpool.tile([128, nt * 8], i16)
            nc.sync.dma_start(idx1_sb[:], idx1_d[:])
            idx2_sb = cpool.tile([128, nt * 8], i16)
            nc.sync.dma_start(idx2_sb[:], idx2_d[:])

            pooled_sb = cpool.tile([g_graphs, H2], f32)
            nc.vector.memset(pooled_sb[:], 0.0)

            qcount = [0]

            def run_layer(tbl_src, idx_sb, self_dram, drain_fn):
                sbuilt = {}
                for sg, blocks in enumerate(groups):
                    gbs = {}
                    for q in range(QN):
                        ctq = int(sched.ct[q, sg])
                        if ctq == 0:
                            continue
                        ecol = int(sched.gstart[q, sg]) // 16
                        gb = gbufp.tile([128, maxct, 128], bf16, tag="gb")
                        nc.gpsimd.dma_gather(
                            gb[:, 0:ctq, :],
                            tbl_src[q * QROWS : (q + 1) * QROWS, :],
                            idx_sb[:, ecol : ecol + ctq * 8],
                            ctq * 128, ctq * 128, 128,
                            single_packet=False,
                            queue_num=qcount[0] % 4,
                        )
                        qcount[0] += 1
                        gbs[q] = gb
                    for bi, b in enumerate(blocks):
                        insts = sched.inst[(sg, b)]
                        alist = [(q, bi * ALIGN_T + a)
                                 for q in range(3) for a in range(ALIGN_T)]
                        total = len(alist) + len(insts)
                        pm = psum.tile([128, 128], f32, tag="agg", bufs=5)
                        selfrow = dr.tile([128, 128], bf16, tag="selfrow")
                        nc.sync.dma_start(
                            selfrow[:], self_dram[b * 128 : (b + 1) * 128, :]
                        )
                        nc.tensor.matmul(
                            pm[:], selfrow[:], ident[:],
                            start=True, stop=(total == 0),
                        )
                        j = 0
                        for q, t in alist:
                            j += 1
                            nc.tensor.matmul(
                                pm[:], gbs[q][:, t, :], ident[:],
                                start=False, stop=(j == total),
                            )
                        for q, t, iid in insts:
                            kb = iid // K
                            if kb not in sbuilt:
                                S = sp.tile([128, K * 128], bf16, tag="S",
                                            bufs=12)
                                s3 = S[:].rearrange("p (k c) -> p k c", k=K)
                                nc.vector.tensor_tensor(
                                    s3,
                                    iota[:].unsqueeze(1).to_broadcast(
                                        (128, K, 128)),
                                    slotc[:, kb * K : (kb + 1) * K]
                                    .unsqueeze(2).to_broadcast((128, K, 128)),
                                    mybir.AluOpType.is_equal,
                                )
                                sbuilt[kb] = S
                            o = (iid % K) * 128
                            j += 1
                            nc.tensor.matmul(
                                pm[:], gbs[q][:, t, :],
                                sbuilt[kb][:, o : o + 128],
                                start=False, stop=(j == total),
                            )
                        drain_fn(b, pm)

            def drain1(b, pm):
                aggT = dr.tile([128, 128], bf16, tag="aggT")
                nc.scalar.activation(
                    aggT[:], pm[:], mybir.ActivationFunctionType.Copy
                )
                pz = psum.tile([128, F], f32, tag="pz", bufs=2)
                nc.tensor.matmul(pz[:], aggT[:], w1[:], start=True, stop=True)
                u = dr.tile([128, F], f32, tag="u")
                nc.vector.scalar_tensor_tensor(
                    u[:], pz[:], dinvc[:, b : b + 1], b1bc[:],
                    mybir.AluOpType.mult, mybir.AluOpType.add,
                )
                hs = stage.tile([128, F], bf16, tag="hs")
                nc.scalar.activation(
                    hs[:], u[:], mybir.ActivationFunctionType.Relu,
                    scale=dinvc[:, b : b + 1],
                )
                nc.sync.dma_start(tbl2_loc[b * 128 : (b + 1) * 128, :], hs[:])

            def drain2(b, pm):
                a2T = dr.tile([128, 128], bf16, tag="aggT")
                nc.scalar.activation(
                    a2T[:], pm[:], mybir.ActivationFunctionType.Copy
                )
                pz = psum.tile([128, H2], f32, tag="pz", bufs=2)
                nc.tensor.matmul(pz[:], a2T[:], w2[:], start=True, stop=True)
                a2 = dr.tile([128, H2], bf16, tag="a2")
                nc.scalar.activation(
                    a2[:], pz[:], mybir.ActivationFunctionType.Copy,
                    scale=dinvc[:, b : b + 1],
                )
                spool = sp.tile([128, g_graphs], bf16, tag="spool")
                nc.vector.tensor_scalar(
                    spool[:], iota[:, 0:g_graphs], batchc[:, b : b + 1],
                    None, mybir.AluOpType.is_equal,
                )
                pp = psum.tile([g_graphs, H2], f32, tag="pp", bufs=1)
                nc.tensor.matmul(pp[:], spool[:], a2[:], start=True, stop=True)
                nc.vector.tensor_add(pooled_sb[:], pooled_sb[:], pp[:])

            run_layer(xtbl, idx1_sb, xself, drain1)
            for gi in range(10):
                l0, rows = T2_GL0[gi], T2_GRPC[gi]
                off = T2_GOFF[gi]
                nc.gpsimd.collective_compute(
                    "AllGather", mybir.AluOpType.bypass, replica_groups=rg,
                    ins=[tbl2_loc[l0 : l0 + rows, :]],
                    outs=[tbl2_ag[off : off + rows * NCORES, :]],
                )
            run_layer(tbl2_ag, idx2_sb, tbl2_loc, drain2)

            nc.sync.dma_start(out_d[:], pooled_sb[:])

    nc.compile()
    return nc


def _install_trace_hooks():
    """Register the axon NTFF profile hook (missing antenv.axon_hooks shim)
    and neuter the artifact upload. Dev/profiling only (K_TRACE_DIR)."""
    import types
    import ctypes
    import contextlib

    if "antenv.axon_hooks" in sys.modules:
        return
    lib = ctypes.CDLL("/opt/axon/libaxon_pjrt.so")
    lib.axon_start_nrt_profile.argtypes = [
        ctypes.POINTER(ctypes.c_int64), ctypes.c_size_t,
    ]
    lib.axon_start_nrt_profile.restype = ctypes.c_int64
    lib.axon_stop_nrt_profile.argtypes = [ctypes.c_char_p]
    lib.axon_stop_nrt_profile.restype = ctypes.c_int64

    @contextlib.contextmanager
    def _hook(output_dir, device_ids):
        import jax
        jax.devices()
        if device_ids:
            ids = (ctypes.c_int64 * len(device_ids))(*device_ids)
            rc = lib.axon_start_nrt_profile(ids, len(device_ids))
        else:
            rc = lib.axon_start_nrt_profile(None, 0)
        if rc != 0:
            raise RuntimeError(f"axon_start_nrt_profile rc={rc}")
        try:
            yield
        finally:
            nfiles = lib.axon_stop_nrt_profile(str(output_dir).encode())
            print(f"ntff profile: {nfiles} file(s) -> {output_dir}")

    mod = types.ModuleType("antenv.axon_hooks")
    mod.get_axon_ntff_profile_hook = lambda: _hook
    mod.set_axon_ntff_profile_hook = lambda h: None
    sys.modules["antenv.axon_hooks"] = mod
    import concourse.bass_utils as _bu
    _bu.upload_artifacts = lambda tmpdir: "local://" + str(tmpdir)


def _prep_and_run(x, src, dst, batch, W1, b1, W2, b2, Wfc, bfc, n, g_graphs):
    global LAST_EXEC_S, LAST_PREP_S, LAST_COMPILE_S
    t0 = time.perf_counter()
    npc = n // NCORES
    npcp = _ceil(npc, 128) * 128
    nb = npcp // 128

    deg = (np.bincount(dst, minlength=n) + 1.0).astype(np.float32)
    dinv = (1.0 / np.sqrt(deg)).astype(np.float32)

    sched, gidx1, gidx2, slot_cols = _schedule(src, dst, npc, nb)
    nt = sched.nt
    maxct = int(sched.ct.max())
    ninst_pad = max(K, _ceil(sched.ninst, K) * K)
    if os.environ.get("K_VERBOSE"):
        print(f"schedule: nt={nt} tiles, ninst={sched.ninst}, maxct={maxct}, "
              f"padded_idx={sched.ntot}, "
              f"real_edges/core={len(dst) / NCORES:.0f}, "
              f"pad_overhead={sched.ntot * NCORES / len(dst) - 1:.1%}")

    idx1_maps = _wrap_idx(sched, gidx1)
    idx2_maps = _wrap_idx(sched, gidx2)

    slots_pad = np.full((NCORES, 128, ninst_pad), -1.0, np.float32)
    slots_pad[:, :, : sched.ninst] = slot_cols

    dinv_pad = np.ones((NCORES, npcp), np.float32)
    batch_pad = np.full((NCORES, npcp), -1.0, np.float32)
    xs_pad = np.zeros((NCORES, npcp, F), ml_dtypes.bfloat16)
    xs = x * dinv[:, None]
    for c in range(NCORES):
        lo, hi = c * npc, (c + 1) * npc
        dinv_pad[c, :npc] = dinv[lo:hi]
        batch_pad[c, :npc] = batch[lo:hi].astype(np.float32)
        xs_pad[c, :npc] = xs[lo:hi].astype(ml_dtypes.bfloat16)
    dinv_cols = dinv_pad.reshape(NCORES, nb, 128).transpose(0, 2, 1).copy()
    batch_cols = batch_pad.reshape(NCORES, nb, 128).transpose(0, 2, 1).copy()

    # full x table in tbl1 (quarter-core-major) layout; zero gaps included
    xtbl = np.zeros((ALLOC_ROWS, F), ml_dtypes.bfloat16)
    for c in range(NCORES):
        for q in range(QN):
            l0, rpc = T1_QL0[q], T1_QRPC[q]
            xtbl[q * QROWS + c * rpc : q * QROWS + (c + 1) * rpc] = \
                xs_pad[c, l0 : l0 + rpc]

    iota = np.broadcast_to(np.arange(128), (128, 128)).astype(ml_dtypes.bfloat16)
    ident = np.eye(128, dtype=ml_dtypes.bfloat16)
    b1bc = np.broadcast_to(b1, (128, F)).astype(np.float32)
    w1b = W1.astype(ml_dtypes.bfloat16)
    w2b = W2.astype(ml_dtypes.bfloat16)

    LAST_PREP_S = time.perf_counter() - t0

    key = (n, g_graphs, npcp, sched.key())
    t0 = time.perf_counter()
    if key not in _BUILD_CACHE:
        _BUILD_CACHE.clear()
        _BUILD_CACHE[key] = _build_bass(
            npcp, nb, sched, ninst_pad, maxct, g_graphs
        )
    nc = _BUILD_CACHE[key]
    LAST_COMPILE_S = time.perf_counter() - t0

    in_maps = []
    for c in range(NCORES):
        in_maps.append(
            {
                "xtbl": xtbl,
                "xself": np.ascontiguousarray(xs_pad[c]),
                "idx1": np.ascontiguousarray(idx1_maps[c]),
                "idx2": np.ascontiguousarray(idx2_maps[c]),
                "slots": np.ascontiguousarray(
                    slots_pad[c].astype(ml_dtypes.bfloat16)
                ),
                "dinvc": np.ascontiguousarray(dinv_cols[c]),
                "batchc": np.ascontiguousarray(batch_cols[c]),
                "b1bc": b1bc,
                "iota": iota,
                "ident": ident,
                "w1": w1b,
                "w2": w2b,
            }
        )
    t0 = time.perf_counter()
    trace_dir = os.environ.get("K_TRACE_DIR")
    if trace_dir:
        _install_trace_hooks()
        res = run_bass_kernel_spmd(
            nc, in_maps, list(range(NCORES)), trace=True, tmpdir=trace_dir
        )
        globals()["LAST_RESULTS"] = res
    else:
        res = run_bass_kernel_spmd(nc, in_maps, list(range(NCORES)))
    LAST_EXEC_S = time.perf_counter() - t0

    pooled = np.zeros((g_graphs, H2), np.float64)
    for c in range(NCORES):
        pooled += res.results[c]["pooled"].astype(np.float64)
    cnt = np.bincount(batch, minlength=g_graphs).astype(np.float64)
    gmean = pooled / np.maximum(cnt, 1.0)[:, None]
    out = (gmean + b2.astype(np.float64)) @ Wfc.astype(np.float64) + bfc.astype(
        np.float64
    )
    return out.astype(np.float32)


def kernel(x, edge_index, batch, W1, b1, W2, b2, Wfc, bfc):
    x = np.asarray(x, dtype=np.float32)
    ei = np.asarray(edge_index)
    src = ei[0].astype(np.int64)
    dst = ei[1].astype(np.int64)
    bat = np.asarray(batch).astype(np.int64)
    n = x.shape[0]
    g_graphs = 64
    return _prep_and_run(
        x, src, dst, bat,
        np.asarray(W1, np.float32), np.asarray(b1, np.float32),
        np.asarray(W2, np.float32), np.asarray(b2, np.float32),
        np.asarray(Wfc, np.float32), np.asarray(bfc, np.float32),
        n, g_graphs,
    )
